# revision 1
# baseline (speedup 1.0000x reference)
# Trainium2 Bass kernel for nn_EquivariantCorrectionHead (v3).
#
# Math (per sample b):
#   s (64,), t (5,5) [v,i]
#   alpha_v = sum_w M2[w,v] h_s_w  folded on host:
#     alpha_s = q-form of s with S~_v = sum_w M2[w,v]*sym(W1sss[:,:,w])  (PE+DVE)
#     alpha_t = WtG @ Gvec, G = t t^T                                    (PE)
#   h_t[(k,w)] = PW1_2*(INV_S5*(stt+tst) + ttt via C)                    (PE+DVE)
#   out_k = sum_ij C_ijk Q_ij + sum_v alpha_v ht[(k,v)],
#     Q = g2 ht^T with g2 = PW2_2 W2ttt-map(ht)
#
# Engine split per 128-sample tile:
#   PE: all matmuls/transposes (z~ chunks, a~, Eb, GT/WtG/alpha-t, ttt blockdiag,
#       ht1 transpose-accum, htb/g2 transposes).
#   ACT: psum f32 -> sbuf bf16 copies (z~, ah, Eh, MT, htTA/B, htb, GT, alpha-t).
#   DVE: 2x-mode multiplies (q1,q2,q7,qq,qg,q10,q12), tree-reduces (alpha-s L1/L2,
#        M fold-5), small 1x reduces (G, o1o2).
#   GPSIMD: pool_avg segmented reduces (alpha-s tail, ht1 fold-5, Q fold-32).
# Data parallel over 8 cores (batch sharded, weights replicated).

import os
import sys
from contextlib import ExitStack

import numpy as np

if "/opt/trn_rl_repo" not in sys.path:
    sys.path.insert(0, "/opt/trn_rl_repo")

import concourse.bass as bass
import concourse.mybir as mybir
import concourse.tile as tile
from concourse import bacc, masks
from concourse.bass_utils import run_bass_kernel_spmd

B, NS, H = 131072, 64, 32
NCORES = 8
BPC = B // NCORES          # 16384 rows per core
P = 128                    # samples per tile
NT_FULL = BPC // P         # 128 tiles per core

PW1_0 = float((NS * NS + 25.0) ** -0.5)
PW1_2 = float((5.0 / (10.0 * NS + 25.0)) ** 0.5)
PW2_2 = float((5.0 / (3.0 * H * H)) ** 0.5)
INV_S5 = float(5.0 ** -0.5)

F32 = mybir.dt.float32
F16 = mybir.dt.float16
AX = mybir.AxisListType
OP = mybir.AluOpType
PF = mybir.PoolFunctionType


def _wigner3j_222():
    s2, s6 = np.sqrt(2.0), np.sqrt(6.0)
    M = np.zeros((5, 3, 3))
    M[0] = np.array([[0.0, 1, 0], [1, 0, 0], [0, 0, 0]]) / s2
    M[1] = np.array([[0.0, 0, 0], [0, 0, 1], [0, 1, 0]]) / s2
    M[2] = np.diag([-1.0, -1, 2]) / s6
    M[3] = np.array([[0.0, 0, 1], [0, 0, 0], [1, 0, 0]]) / s2
    M[4] = np.diag([1.0, -1, 0]) / s2
    C = np.einsum("aij,bjk,cki->abc", M, M, M)
    C = 0.5 * (C + C.transpose(1, 0, 2))
    return (C / np.linalg.norm(C)).astype(np.float64)


def prep_weights(w1_sss, w1_stt, w1_tst, w1_tts, w1_ttt, w2_stt, w2_tst, w2_ttt):
    """Host-side weight preprocessing. Returns dict of device const arrays (f16)."""
    C = _wigner3j_222()
    w1_sss = np.asarray(w1_sss, np.float64)
    w1_stt = np.asarray(w1_stt, np.float64)
    w1_tst = np.asarray(w1_tst, np.float64)
    w1_tts = np.asarray(w1_tts, np.float64)
    w1_ttt = np.asarray(w1_ttt, np.float64)
    w2_stt = np.asarray(w2_stt, np.float64)[:, :, 0]
    w2_tst = np.asarray(w2_tst, np.float64)[:, :, 0]
    w2_ttt = np.asarray(w2_ttt, np.float64)[:, :, 0]

    # alpha map M2 [w_hidden(u), v]: alpha_v = sum_u M2[u,v] h_s_u
    M2 = (PW2_2 * INV_S5) * (w2_stt + w2_tst.T)      # [32, 32]

    # --- alpha_s: fold M2 into the sss weights ---
    # h_s_w(s-part) = PW1_0 * sum_uv W1sss[u,v,w] s_u s_v
    # alpha_s[a] = sum_w M2[w,a] h_s_w = sum_uv Stil[u,v,a] s_u s_v
    Stil = PW1_0 * np.einsum("uvw,wa->uva", w1_sss, M2)           # [64,64,32]
    # Device: zt[b,(a,v)] = sum_u s_u Stil[u,v,a]; q1 = zt * s_v; tree-sum v
    # Wzt [64, (a,v)=2048], cols a-major v-minor
    Wzt = np.transpose(Stil, (0, 2, 1)).reshape(NS, H * NS)       # u,(a,v)

    # --- alpha_t: WtG[(u,v)=25, a=32] : alpha_t[a] = sum_uv G_uv WtG[(uv),a] ---
    # h_s_w(t-part) = PW1_0*INV_S5 * sum_uv W1tts[u,v,w] G_uv
    WtG = (PW1_0 * INV_S5) * np.einsum("uvw,wa->uva", w1_tts, M2).reshape(25, H)

    # --- a~ (stt/tst combined): A[w,v] = sum_u s_u * Wcomb[u,v,w] ---
    # device layout ah [b,(w,v)]: cols w-major v-minor (w*5+v)
    Wa = (PW1_2 * INV_S5) * (
        np.transpose(w1_stt, (0, 2, 1)) + np.transpose(w1_tst, (1, 2, 0))
    ).reshape(NS, H * 5)  # u,(w,v)

    # --- E-step lhsT: Cbig [(v',i)=25, (k,u,j)=125] ---
    # E[b,(k,u,j)] = sum_i C[i,j,k] t[b,(u,i)]  (per u: delta_{u,v'})
    Cbig = np.zeros((25, 125))
    for k in range(5):
        for u in range(5):
            for j in range(5):
                for i in range(5):
                    Cbig[u * 5 + i, k * 25 + u * 5 + j] = C[i, j, k]

    # ttt block-diag lhsT: [(k,u,v)=125, (k',w)]: d_{kk'} * PW1_2*W1ttt[u,v,w]
    wttt = PW1_2 * w1_ttt.reshape(25, H)  # (u,v),w
    WtttA = np.zeros((125, 128))  # k'=0..3
    WtttB = np.zeros((125, 32))   # k'=4
    for k in range(4):
        WtttA[k * 25 : k * 25 + 25, k * 32 : k * 32 + 32] = wttt
    WtttB[100:125, :] = wttt

    # g2 block-diag lhsT: [(i,u), (i',v)]: d_{ii'} * PW2_2*W2ttt[u,v]
    w2t = PW2_2 * w2_ttt
    W2A = np.zeros((128, 128))  # i=0..3
    for i in range(4):
        W2A[i * 32 : i * 32 + 32, i * 32 : i * 32 + 32] = w2t
    W2B = w2t.copy()  # i=4, [32,32]

    # C replicated for o1: [128, (k,c)=(5,25)], val C[i,j,k] at c=(i,j)
    crep = np.transpose(C, (2, 0, 1)).reshape(1, 125)
    Crep2 = np.broadcast_to(crep, (P, 125)).copy()

    return {
        "Wzt": np.ascontiguousarray(Wzt, np.float16),
        "Wa": np.ascontiguousarray(Wa, np.float16),
        "WtG": np.ascontiguousarray(WtG, np.float16),
        "Cbig": np.ascontiguousarray(Cbig, np.float16),
        "WtttA": np.ascontiguousarray(WtttA, np.float16),
        "WtttB": np.ascontiguousarray(WtttB, np.float16),
        "W2A": np.ascontiguousarray(W2A, np.float16),
        "W2B": np.ascontiguousarray(W2B, np.float16),
        "Crep2": np.ascontiguousarray(Crep2, np.float16),
    }


WEIGHT_SHAPES = {
    "Wzt": (NS, H * NS),
    "Wa": (NS, H * 5),
    "WtG": (25, H),
    "Cbig": (25, 125),
    "WtttA": (125, 128),
    "WtttB": (125, 32),
    "W2A": (128, 128),
    "W2B": (32, 32),
    "Crep2": (P, 125),
}


def _tile_body(ctx: ExitStack, tc: tile.TileContext, io, n_tiles: int):
    nc = tc.nc
    s_d, t_d, tk_d, out_d, wd = io["s"], io["t"], io["tk"], io["out"], io["w"]
    sT_d, tT_d = io["sT"], io["tT"]

    const = ctx.enter_context(tc.tile_pool(name="const", bufs=1))
    W = {}
    for name, shp in WEIGHT_SHAPES.items():
        W[name] = const.tile(list(shp), F16, tag=name, name=f"W_{name}")
        nc.sync.dma_start(W[name][:], wd[name])
    ident = const.tile([128, 128], F16, tag="ident")
    masks.make_identity(nc, ident[:])
    ident32 = const.tile([128, 128], F32, tag="ident32")
    masks.make_identity(nc, ident32[:])

    io_pool = ctx.enter_context(tc.tile_pool(name="io", bufs=3))
    sb = ctx.enter_context(tc.tile_pool(name="sb", bufs=2))
    qb = ctx.enter_context(tc.tile_pool(name="qb", bufs=2))
    zps = ctx.enter_context(tc.tile_pool(name="zps", bufs=2, space="PSUM"))
    aps_ = ctx.enter_context(tc.tile_pool(name="aps", bufs=1, space="PSUM"))
    tps = ctx.enter_context(tc.tile_pool(name="tps", bufs=2, space="PSUM"))

    ctx.enter_context(nc.allow_low_precision("fp16 intermediates fit the 2e-2 budget"))
    for it in range(n_tiles):
        r0 = it * P
        # --- load inputs ---
        st = io_pool.tile([P, 114], F16, tag="st")
        nc.sync.dma_start(st[:, 0:64], s_d[r0 : r0 + P, :])
        nc.sync.dma_start(st[:, 64:89], t_d[r0 : r0 + P, :])
        nc.sync.dma_start(st[:, 89:114], tk_d[r0 : r0 + P, :])
        s_sb = st[:, 0:64]
        t_sb = st[:, 64:89]     # t[v,i] layout (v-major, i-minor)
        tk_sb = st[:, 89:114]   # t^T: [k, v] layout (k-major, v-minor)

        sT = sb.tile([64, P], F16, tag="sT")
        nc.sync.dma_start(sT[:], sT_d[:, r0 : r0 + P])
        tT = sb.tile([25, P], F16, tag="tT")
        nc.sync.dma_start(tT[:], tT_d[:, r0 : r0 + P])

        # ================= alpha_s: zt = s @ Wzt; q1 = zt*s; sum over v ======
        # four psum chunks [P,512] f32; ACT casts into one zh, one big DVE mult
        q1 = qb.tile([P, H * NS], F16, tag="q1")
        zh = qb.tile([P, H * NS], F16, tag="zh")
        for hc in range(4):
            zc = zps.tile([P, 512], F32, tag="zc")
            nc.tensor.matmul(
                zc[:], sT[:], W["Wzt"][:, hc * 512 : hc * 512 + 512],
                start=True, stop=True,
            )
            nc.scalar.copy(zh[:, hc * 512 : hc * 512 + 512], zc[:])
        nc.vector.tensor_tensor(
            q1[:].rearrange("p (a v) -> p a v", a=H),
            zh[:].rearrange("p (a v) -> p a v", a=H),
            s_sb.unsqueeze(1).to_broadcast((P, H, 64)),
            OP.mult,
        )
        # tree reduce over v: L1 adds halves (64->32, 2x mode), pool does 32->1
        q1t = qb.tile([P, H * 32], F16, tag="q1t")
        q1v = q1[:].rearrange("p (a v) -> p a v", a=H)
        q1tv = q1t[:].rearrange("p (a v) -> p a v", a=H)
        nc.vector.tensor_tensor(q1tv, q1v[:, :, 0:32], q1v[:, :, 32:64], OP.add)
        q1u = qb.tile([P, H * 16], F16, tag="q1u")
        q1uv = q1u[:].rearrange("p (a v) -> p a v", a=H)
        nc.vector.tensor_tensor(q1uv, q1tv[:, :, 0:16], q1tv[:, :, 16:32], OP.add)
        als = sb.tile([P, H], F16, tag="als")
        nc.vector.tensor_reduce(als[:], q1uv, axis=AX.X, op=OP.add)

        # ================= a~ : A[w,v] = s @ Wa =============================
        a_ps = aps_.tile([P, H * 5], F32, tag="a")
        nc.tensor.matmul(a_ps[:], sT[:], W["Wa"][:], start=True, stop=True)
        ah = sb.tile([P, H * 5], F16, tag="ah")
        nc.scalar.copy(ah[:], a_ps[:])

        # q2[(k,w,v)] = A[w,v] * t[v,k] ; ht1[(k,w)] = sum_v q2
        q2 = qb.tile([P, 5 * H * 5], F16, tag="q2")
        q2v = q2[:].rearrange("p (k w v) -> p k w v", k=5, w=H)
        a3 = ah[:].rearrange("p (w v) -> p w v", w=H).unsqueeze(1).to_broadcast((P, 5, H, 5))
        t_kv = tk_sb.rearrange("p (k v) -> p k v", k=5).unsqueeze(2).to_broadcast((P, 5, H, 5))
        nc.vector.tensor_tensor(q2v, a3, t_kv, OP.mult)
        q2t = qb.tile([P, 5 * H * 2], F16, tag="q2t")
        q2tv = q2t[:].rearrange("p (c j) -> p c j", c=5 * H)
        nc.vector.tensor_tensor(
            q2tv,
            q2v[:, :, :, 0:2].rearrange("p k w v -> p (k w) v"),
            q2v[:, :, :, 2:4].rearrange("p k w v -> p (k w) v"),
            OP.add,
        )
        ht1h = qb.tile([P, 5 * H], F16, tag="ht1h")
        nc.vector.tensor_tensor(ht1h[:], q2tv[:, :, 0], q2tv[:, :, 1], OP.add)
        ht1 = sb.tile([P, 5 * H], F32, tag="ht1")
        nc.vector.tensor_tensor(
            ht1[:].rearrange("p (c o) -> p c o", o=1),
            ht1h[:].rearrange("p (c o) -> p c o", o=1),
            q2v[:, :, :, 4:5].rearrange("p k w v -> p (k w) v"),
            OP.add,
        )

        # ================= gram G[u,v] = sum_i t_ui t_vi =====================
        qg = qb.tile([P, 125], F16, tag="qg")
        qgv = qg[:].rearrange("p (u v i) -> p u v i", u=5, v=5)
        t_ui = t_sb.rearrange("p (u i) -> p u i", u=5).unsqueeze(2).to_broadcast((P, 5, 5, 5))
        t_vi = t_sb.rearrange("p (v i) -> p v i", v=5).unsqueeze(1).to_broadcast((P, 5, 5, 5))
        nc.gpsimd.tensor_tensor(qgv, t_ui, t_vi, OP.mult)
        G = sb.tile([P, 25], F16, tag="G")
        nc.vector.tensor_reduce(
            G[:].rearrange("p (u v) -> p u v", u=5), qgv, axis=AX.X, op=OP.add
        )

        # alpha_t = WtG^T @ G^T : transpose G, then matmul
        GT_ps = tps.tile([25, P], F16, tag="tp")
        nc.tensor.transpose(GT_ps[:], G[:], ident[:])
        GT = sb.tile([25, P], F16, tag="GT")
        nc.scalar.copy(GT[:], GT_ps[:])
        alt_ps = tps.tile([H, P], F32, tag="tp")
        nc.tensor.matmul(alt_ps[:], W["WtG"][:], GT[:], start=True, stop=True)
        altT = sb.tile([H, P], F16, tag="altT")
        nc.scalar.copy(altT[:], alt_ps[:])
        alt_s = tps.tile([P, H], F16, tag="tp")
        nc.tensor.transpose(alt_s[:], altT[:], ident[0:32, 0:32])

        # alpha = als + alpha_t
        alh = sb.tile([P, H], F16, tag="alh")
        nc.vector.tensor_tensor(alh[:], als[:], alt_s[:], OP.add)

        # ================= E[b,(k,u,j)] = tT @ Cbig ==========================
        Eb_ps = tps.tile([P, 125], F32, tag="tp")
        nc.tensor.matmul(Eb_ps[:], tT[:], W["Cbig"][:], start=True, stop=True)
        Eh = sb.tile([P, 125], F16, tag="Eh")
        nc.scalar.copy(Eh[:], Eb_ps[:])

        # q7[(k,u,v,j)] = E[(k,u,j)] * t[v,j] ; M[(k,u,v)] = sum_j q7
        # in0: E view [p,(ku),1->v,j] strides (5,0,1); in1: t view [p,1->(ku),v,j]
        q7 = qb.tile([P, 625], F16, tag="q7")
        q7v = q7[:].rearrange("p (c v j) -> p c v j", c=25, v=5)
        E3 = (
            Eh[:].rearrange("p (c j) -> p c j", c=25).unsqueeze(2).to_broadcast((P, 25, 5, 5))
        )
        t_vj = (
            t_sb.rearrange("p (v j) -> p v j", v=5).unsqueeze(1).to_broadcast((P, 25, 5, 5))
        )
        nc.vector.tensor_tensor(q7v, E3, t_vj, OP.mult)
        # M fold-5 tree on DVE: L1 pairs (2x), then two small adds
        Mt = qb.tile([P, 250], F16, tag="Mt")
        Mtv = Mt[:].rearrange("p (c j) -> p c j", c=125)
        nc.vector.tensor_tensor(
            Mtv,
            q7v[:, :, :, 0:2].rearrange("p c v j -> p (c v) j"),
            q7v[:, :, :, 2:4].rearrange("p c v j -> p (c v) j"),
            OP.add,
        )
        Ma = qb.tile([P, 125], F16, tag="Ma")
        nc.vector.tensor_tensor(Ma[:], Mtv[:, :, 0], Mtv[:, :, 1], OP.add)
        M = sb.tile([P, 125], F16, tag="M")
        nc.vector.tensor_tensor(
            M[:].rearrange("p (c v) -> p c v", c=25),
            Ma[:].rearrange("p (c v) -> p c v", c=25),
            q7v[:, :, :, 4],
            OP.add,
        )

        # ================= ht2 = blockdiag(W1ttt) @ M^T ; + ht1^T ===========
        MT_ps = tps.tile([125, P], F16, tag="tp")
        nc.tensor.transpose(MT_ps[:], M[:], ident[:])
        MT = sb.tile([125, P], F16, tag="MT")
        nc.scalar.copy(MT[:], MT_ps[:])

        htTA_ps = tps.tile([P, P], F32, tag="tpA")
        htTB_ps = tps.tile([32, P], F32, tag="tpB", bufs=1)
        nc.tensor.matmul(htTA_ps[:], W["WtttA"][:], MT[:], start=True, stop=False)
        nc.tensor.matmul(htTB_ps[:], W["WtttB"][:], MT[:], start=True, stop=False)
        nc.tensor.matmul(htTA_ps[:], ht1[:, 0:128], ident32[:], is_transpose=True, start=False, stop=True)
        nc.tensor.matmul(htTB_ps[:], ht1[:, 128:160], ident32[:], is_transpose=True, start=False, stop=True)
        htTA = sb.tile([P, P], F16, tag="htTA")
        nc.scalar.copy(htTA[:], htTA_ps[:])
        htTB = sb.tile([32, P], F16, tag="htTB")
        nc.scalar.copy(htTB[:], htTB_ps[:])

        # ht sample-major [p, (k,w)]
        htb_ps = tps.tile([P, 160], F16, tag="tpA")
        nc.tensor.transpose(htb_ps[:, 0:128], htTA[:], ident[:])
        nc.tensor.transpose(htb_ps[:, 128:160], htTB[:], ident[0:32, 0:32])
        htb = sb.tile([P, 160], F16, tag="htb")
        nc.scalar.copy(htb[:], htb_ps[:])

        # ================= g2 = blockdiag(W2ttt) @ htT =======================
        g2A_ps = tps.tile([P, P], F32, tag="tpA")
        nc.tensor.matmul(g2A_ps[:], W["W2A"][:], htTA[:], start=True, stop=True)
        g2B_ps = tps.tile([32, P], F32, tag="tpB", bufs=1)
        nc.tensor.matmul(g2B_ps[:], W["W2B"][:], htTB[:], start=True, stop=True)
        g2A_sb = sb.tile([P, P], F16, tag="g2A")
        nc.scalar.copy(g2A_sb[:], g2A_ps[:])
        g2B_sb = sb.tile([32, P], F16, tag="g2B")
        nc.scalar.copy(g2B_sb[:], g2B_ps[:])
        g2b_ps = tps.tile([P, 160], F16, tag="tpA")  # [p,(i,v)]
        nc.tensor.transpose(g2b_ps[:, 0:128], g2A_sb[:], ident[:])
        nc.tensor.transpose(g2b_ps[:, 128:160], g2B_sb[:], ident[0:32, 0:32])

        # ================= Q[(i,j)] = sum_v g2[(i,v)] ht[(j,v)] ==============
        qq = qb.tile([P, 800], F16, tag="qq")
        qqv = qq[:].rearrange("p (i j v) -> p i j v", i=5, j=5)
        g2_b = g2b_ps[:].rearrange("p (i v) -> p i v", i=5).unsqueeze(2).to_broadcast((P, 5, 5, 32))
        ht_b = htb[:].rearrange("p (j v) -> p j v", j=5).unsqueeze(1).to_broadcast((P, 5, 5, 32))
        nc.vector.tensor_tensor(qqv, g2_b, ht_b, OP.mult)
        qqt = qb.tile([P, 400], F16, tag="qqt")
        qqtv = qqt[:].rearrange("p (c v) -> p c v", c=25)
        nc.vector.tensor_tensor(
            qqtv,
            qqv[:, :, :, 0:16].rearrange("p i j v -> p (i j) v"),
            qqv[:, :, :, 16:32].rearrange("p i j v -> p (i j) v"),
            OP.add,
        )
        qqu = qb.tile([P, 200], F16, tag="qqu")
        qquv = qqu[:].rearrange("p (c v) -> p c v", c=25)
        nc.vector.tensor_tensor(qquv, qqtv[:, :, 0:8], qqtv[:, :, 8:16], OP.add)
        Q = sb.tile([P, 25], F16, tag="Q")
        nc.vector.tensor_reduce(Q[:], qquv, axis=AX.X, op=OP.add)

        # ================= o1+o2 fused =======================================
        # q1012 [p, (k, 57)]: cols 0:25 = C_k * Q, cols 25:57 = alpha * ht[(k,:)]
        q1012 = qb.tile([P, 285], F16, tag="q1012")
        qv = q1012[:].rearrange("p (k c) -> p k c", k=5)
        nc.gpsimd.tensor_tensor(
            qv[:, :, 0:25],
            Q[:].unsqueeze(1).to_broadcast((P, 5, 25)),
            W["Crep2"][:].rearrange("p (k c) -> p k c", k=5),
            OP.mult,
        )
        nc.vector.tensor_tensor(
            qv[:, :, 25:57],
            alh[:].unsqueeze(1).to_broadcast((P, 5, 32)),
            htb[:].rearrange("p (k v) -> p k v", k=5),
            OP.mult,
        )
        out_sb = io_pool.tile([P, 5], F32, tag="out_sb")
        nc.vector.tensor_reduce(out_sb[:], qv, axis=AX.X, op=OP.add)
        nc.sync.dma_start(out_d[r0 : r0 + P, :], out_sb[:])


def build_program(n_tiles=NT_FULL):
    nc = bacc.Bacc(
        "TRN2",
        target_bir_lowering=False,
        debug=False,
        enable_asserts=False,
        num_devices=NCORES,
    )
    rows = n_tiles * P
    io = {
        "s": nc.dram_tensor("s", [rows, NS], F16, kind="ExternalInput").ap(),
        "t": nc.dram_tensor("t", [rows, 25], F16, kind="ExternalInput").ap(),
        "tk": nc.dram_tensor("tk", [rows, 25], F16, kind="ExternalInput").ap(),
        "sT": nc.dram_tensor("sT", [NS, rows], F16, kind="ExternalInput").ap(),
        "tT": nc.dram_tensor("tT", [25, rows], F16, kind="ExternalInput").ap(),
        "out": nc.dram_tensor("out", [rows, 5], F32, kind="ExternalOutput").ap(),
        "w": {
            name: nc.dram_tensor(name, list(shp), F16, kind="ExternalInput").ap()
            for name, shp in WEIGHT_SHAPES.items()
        },
    }
    with tile.TileContext(nc) as tc:
        with ExitStack() as ctx:
            _tile_body(ctx, tc, io, n_tiles)
    nc.compile()
    return nc


def make_in_maps(
    scalars, kernel_t2_sum, mc_t2, coulomb_t2, bs_t2, mopac_coulomb_t2,
    w1_sss, w1_stt, w1_tst, w1_tts, w1_ttt, w2_stt, w2_tst, w2_ttt,
):
    wmap = prep_weights(w1_sss, w1_stt, w1_tst, w1_tts, w1_ttt, w2_stt, w2_tst, w2_ttt)
    s = np.ascontiguousarray(np.asarray(scalars, np.float16))
    t32 = np.stack(
        [
            np.asarray(kernel_t2_sum, np.float32),
            np.asarray(mc_t2, np.float32),
            np.asarray(coulomb_t2, np.float32),
            np.asarray(bs_t2, np.float32),
            np.asarray(mopac_coulomb_t2, np.float32),
        ],
        axis=1,
    )  # [B, 5(v), 5(i)]
    t = np.ascontiguousarray(t32.reshape(B, 25).astype(np.float16))
    tk = np.ascontiguousarray(np.transpose(t32, (0, 2, 1)).reshape(B, 25).astype(np.float16))
    in_maps = []
    for c in range(NCORES):
        sh = s[c * BPC : (c + 1) * BPC]
        th = t[c * BPC : (c + 1) * BPC]
        m = {
            "s": sh,
            "t": th,
            "tk": tk[c * BPC : (c + 1) * BPC],
            "sT": np.ascontiguousarray(sh.T),
            "tT": np.ascontiguousarray(th.T),
        }
        m.update(wmap)
        in_maps.append(m)
    return in_maps


_CACHED_NC = None


def kernel(
    scalars, kernel_t2_sum, mc_t2, coulomb_t2, bs_t2, mopac_coulomb_t2,
    w1_sss, w1_stt, w1_tst, w1_tts, w1_ttt, w2_stt, w2_tst, w2_ttt,
):
    global _CACHED_NC
    if _CACHED_NC is None:
        _CACHED_NC = build_program(NT_FULL)
    nc = _CACHED_NC

    in_maps = make_in_maps(
        scalars, kernel_t2_sum, mc_t2, coulomb_t2, bs_t2, mopac_coulomb_t2,
        w1_sss, w1_stt, w1_tst, w1_tts, w1_ttt, w2_stt, w2_tst, w2_ttt,
    )
    res = run_bass_kernel_spmd(nc, in_maps, list(range(NCORES)))
    out = np.concatenate([res.results[c]["out"] for c in range(NCORES)], axis=0)
    return out.astype(np.float32)

